# revision 32
# baseline (speedup 1.0000x reference)
"""GQA attention block (RMSNorm-QK + RoPE + causal attention + proj) on 8 TRN2 cores.

Sharding: DP=2 over batch x TP=4 over heads (4 q heads + 1 kv head per core).
Per core: x[b] @ Wq_shard / Wkv_shard -> q,k,v; RMSNorm+RoPE (cos/sin tables
precomputed on host, q_scale/k_scale and 1/sqrt(HS) baked in); causal
flash-ish attention in bf16 with f32 softmax stats; AllGather of y^T over the
4 TP ranks; column-parallel Wproj. Host pre-transposes x so the device never
transposes activations for the projections.
"""

import math
import os
import sys

import numpy as np

for _p in ("/opt/trn_rl_repo", "/root/.axon_site/_ro/trn_rl_repo"):
    if os.path.isdir(_p) and _p not in sys.path:
        sys.path.insert(0, _p)

import ml_dtypes

import concourse.bacc as bacc
import concourse.mybir as mybir
import concourse.tile as tile
from concourse import masks
from concourse.bass_utils import run_bass_kernel_spmd

BF16 = mybir.dt.bfloat16
F32 = mybir.dt.float32
AX = mybir.AxisListType
ALU = mybir.AluOpType
AF = mybir.ActivationFunctionType

B, T, C = 2, 2048, 2048
NH, NKV, HS = 16, 4, 128
TP = 4                # tensor-parallel ranks per batch element
QH = NH // TP         # q heads per core
QW = QH * HS          # 512
PT = 128
NT = T // PT          # 16
NCT = C // PT         # 16
H2 = HS // 2
EPS = 1e-6
THETA = 10000.0
NCORES = 8
BF = ml_dtypes.bfloat16

_CACHE = {}


def _build(loop_r=None, ablate=()):
    nc = bacc.Bacc(None, target_bir_lowering=False, num_devices=NCORES)

    xT = nc.declare_dram_parameter("xT", [C, T], BF16, isOutput=False)
    wq = nc.declare_dram_parameter("wq", [C, QW], BF16, isOutput=False)
    wkv = nc.declare_dram_parameter("wkv", [C, 2 * HS], BF16, isOutput=False)
    wp = nc.declare_dram_parameter("wp", [C, QW], BF16, isOutput=False)
    v1sT = nc.declare_dram_parameter("v1sT", [HS, T], F32, isOutput=False)
    cosqT = nc.declare_dram_parameter("cosqT", [HS, T], BF16, isOutput=False)
    sinqT = nc.declare_dram_parameter("sinqT", [HS, T], BF16, isOutput=False)
    coskT = nc.declare_dram_parameter("coskT", [HS, T], BF16, isOutput=False)
    sinkT = nc.declare_dram_parameter("sinkT", [HS, T], BF16, isOutput=False)
    mneg = nc.declare_dram_parameter("mneg", [PT, PT], F32, isOutput=False)
    out = nc.declare_dram_parameter("out", [T, QW], F32, isOutput=True)

    groups = [[0, 1, 2, 3], [4, 5, 6, 7]]

    with tile.TileContext(nc) as tc:
        with (
            tc.tile_pool(name="const", bufs=1) as const,
            tc.tile_pool(name="persist", bufs=1) as persist,
            tc.tile_pool(name="psum", bufs=1, space="PSUM") as psum,
            tc.tile_pool(name="wk", bufs=3) as wk,
            tc.tile_pool(name="dram", bufs=1, space="DRAM") as dram,
        ):
            ident = const.tile([PT, PT], BF16)
            masks.make_identity(nc, ident[:])
            maskt = const.tile([PT, PT], F32)
            nc.sync.dma_start(maskt[:], mneg[:])
            eps_t = const.tile([PT, 1], F32)
            nc.gpsimd.memset(eps_t[:], EPS)
            ones_t = const.tile([PT, 1], BF16)
            nc.gpsimd.memset(ones_t[:], 1.0)
            ones1 = const.tile([1, PT], BF16)
            nc.gpsimd.memset(ones1[:], 1.0)

            wq_s = persist.tile([PT, NCT, QW], BF16)
            wkv_s = persist.tile([PT, NCT, 2 * HS], BF16)
            wp_s = persist.tile([PT, NCT, QW], BF16)
            for ci in range(NCT):
                nc.sync.dma_start(wq_s[:, ci, :], wq[ci * PT:(ci + 1) * PT, :])
                nc.sync.dma_start(wkv_s[:, ci, :], wkv[ci * PT:(ci + 1) * PT, :])
                nc.sync.dma_start(wp_s[:, ci, :], wp[ci * PT:(ci + 1) * PT, :])

            qT_s = persist.tile([PT, QH, T], BF16)
            kT_s = persist.tile([PT, T], BF16)
            v_s = persist.tile([PT, NT, HS + 1], BF16)
            nc.gpsimd.memset(v_s[:, :, HS:HS + 1], 1.0)
            yT_s = persist.tile([PT, QH, T], BF16)

            TC = 256
            NCH = T // TC
            ag_ins = [dram.tile([QW, TC], BF16, name=f"ag_in{c}") for c in range(NCH)]
            ag_outs = [dram.tile([C, TC], BF16, name=f"ag_out{c}") for c in range(NCH)]

            def _load_xt(xt_s):
                for ci in range(NCT):
                    nc.sync.dma_start(xt_s[:, ci, :], xT[ci * PT:(ci + 1) * PT, :])

            def _ag_issue(c):
                # gather chunk c of y^T across the TP group
                c0 = c * TC
                nc.sync.dma_start(
                    ag_ins[c][:].rearrange("(h p) t -> p h t", p=PT),
                    yT_s[:, :, c0:c0 + TC],
                )
                nc.gpsimd.collective_compute(
                    "AllGather", ALU.bypass, replica_groups=groups,
                    ins=[ag_ins[c][:]], outs=[ag_outs[c][:]],
                )

            def _proj_chunk(c, src, local):
                # project t-chunk c: 4 row-tiles of 128
                for tt in range(TC // PT):
                    ti = c * (TC // PT) + tt
                    lt0 = (tt if local else ti) * PT
                    pp = psum.tile([PT, QW], F32, tag="b", bufs=2)
                    for ci in range(NCT):
                        nc.tensor.matmul(
                            pp[:], src[:, ci, lt0:lt0 + PT], wp_s[:, ci, :],
                            start=(ci == 0), stop=(ci == NCT - 1),
                        )
                    ot = wk.tile([PT, QW], F32, tag="ot", bufs=2)
                    nc.vector.tensor_copy(ot[:], pp[:])
                    nc.sync.dma_start(out[ti * PT:(ti + 1) * PT, :], ot[:])

            def _stage23(xt_s, proj_cb=None):
                skip = ablate
                NCHK = T // 512
                ZTAGS = ("a", "b", "c", "d")

                # Phase A': weights-stationary QKV -> q^T/k^T/v^T chunks.
                # units: 0..3 q heads, 4 = k, 5 = v
                for u in (() if "A" in skip else range(QH + 2)):
                    if u < QH:
                        wt, w0 = wq_s, u * HS
                    elif u == QH:
                        wt, w0 = wkv_s, 0
                    else:
                        wt, w0 = wkv_s, HS
                    zts = [psum.tile([PT, 512], F32, tag=ZTAGS[ch], bufs=2,
                                     name=f"zt{u}_{ch}")
                           for ch in range(NCHK)]
                    for ci in range(NCT):
                        for ch in range(NCHK):
                            nc.tensor.matmul(
                                zts[ch][:], wt[:, ci, w0:w0 + HS],
                                xt_s[:, ci, ch * 512:(ch + 1) * 512],
                                start=(ci == 0), stop=(ci == NCT - 1),
                            )
                    for ch in range(NCHK):
                        zT = zts[ch]
                        c0 = ch * 512
                        if u == QH + 1:
                            # v: residual mix then transpose to natural layout
                            v1tt = wk.tile([PT, 512], F32, tag="v1tt", bufs=2)
                            nc.sync.dma_start(v1tt[:], v1sT[:, c0:c0 + 512])
                            vmx = wk.tile([PT, 512], BF16, tag="vmx", bufs=2)
                            nc.vector.tensor_tensor(vmx[:], zT[:], v1tt[:], ALU.add)
                            for b in range(4):
                                j = ch * 4 + b
                                tv = psum.tile([PT, PT], BF16, tag="a", bufs=2,
                                               name=f"tv{u}_{ch}_{b}")
                                nc.tensor.transpose(
                                    tv[:], vmx[:, b * PT:(b + 1) * PT], ident[:]
                                )
                                nc.vector.tensor_copy(v_s[:, j, 0:HS], tv[:])
                            continue
                        # q/k: stats -> rsqrt -> rope -> scale
                        sqT = wk.tile([PT, 512], BF16, tag="sqT", bufs=2)
                        nc.scalar.square(sqT[:], zT[:])
                        msT = psum.tile([1, 512], F32, tag="b", bufs=2,
                                        name=f"ms{u}_{ch}")
                        nc.tensor.matmul(msT[:], ones_t[:], sqT[:],
                                         start=True, stop=True)
                        rsT = wk.tile([1, 512], BF16, tag="rsT", bufs=2)
                        nc.scalar.activation(rsT[:], msT[:], AF.Abs_reciprocal_sqrt,
                                             bias=eps_t[0:1, :], scale=1.0 / HS)
                        rsB = psum.tile([PT, 512], F32, tag="b", bufs=2,
                                        name=f"rsB{u}_{ch}")
                        nc.tensor.matmul(rsB[:], ones1[:], rsT[:],
                                         start=True, stop=True)

                        ct, st = (cosqT, sinqT) if u < QH else (coskT, sinkT)
                        cqtT = wk.tile([PT, 512], BF16, tag="cqtT", bufs=2)
                        nc.sync.dma_start(cqtT[:], ct[:, c0:c0 + 512])
                        sqtT = wk.tile([PT, 512], BF16, tag="sqtT", bufs=2)
                        nc.sync.dma_start(sqtT[:], st[:, c0:c0 + 512])

                        zrot = wk.tile([PT, 512], BF16, tag="zrot", bufs=2)
                        nc.vector.tensor_scalar_mul(zrot[0:H2, :], zT[H2:HS, :], -1.0)
                        nc.vector.tensor_copy(zrot[H2:HS, :], zT[0:H2, :])
                        zcos = wk.tile([PT, 512], BF16, tag="zcos", bufs=2)
                        nc.vector.tensor_tensor(zcos[:], zT[:], cqtT[:], ALU.mult)
                        nc.vector.tensor_tensor(zrot[:], zrot[:], sqtT[:], ALU.mult)
                        nc.vector.tensor_tensor(zcos[:], zcos[:], zrot[:], ALU.add)
                        dst = qT_s[:, u, c0:c0 + 512] if u < QH else kT_s[:, c0:c0 + 512]
                        nc.vector.tensor_tensor(dst, zcos[:], rsB[:], ALU.mult)

                # Phase C: causal attention, row QUADS (ACT: Exp only).
                # s^T = k^T(stat) . q^T(mov, 512 wide); exp -> p^T in SBUF;
                # y+rowsum from one matmul vs v_aug (ones column).
                YP_TAGS = ("c", "d", "c", "d")
                for qi in (() if "C" in skip else range(NT // 4)):
                    t0r = 4 * qi
                    q0 = t0r * PT
                    for h in range(QH):
                        yps = [psum.tile([PT, HS + 1], F32, tag=YP_TAGS[r], bufs=2,
                                         name=f"yp{qi}_{h}_{r}")
                               for r in range(4)]
                        for j in range(t0r):
                            spT = psum.tile([PT, 4 * PT], F32, tag="a", bufs=2)
                            nc.tensor.matmul(
                                spT[:], kT_s[:, j * PT:(j + 1) * PT],
                                qT_s[:, h, q0:q0 + 4 * PT], start=True, stop=True,
                            )
                            pts = wk.tile([PT, 4 * PT], BF16, tag="pts", bufs=3)
                            nc.scalar.activation(pts[:], spT[:], AF.Exp)
                            for r in range(4):
                                nc.tensor.matmul(
                                    yps[r][:], pts[:, r * PT:(r + 1) * PT],
                                    v_s[:, j, :],
                                    start=(j == 0), stop=False,
                                )
                        # ragged diagonal block: j = t0r+d covers rows d..3
                        for d in range(4):
                            j = t0r + d
                            w = (4 - d) * PT
                            spT = psum.tile([PT, 4 * PT], F32, tag="a", bufs=2)
                            nc.tensor.matmul(
                                spT[:, 0:w], kT_s[:, j * PT:(j + 1) * PT],
                                qT_s[:, h, j * PT:q0 + 4 * PT],
                                start=True, stop=True,
                            )
                            nc.vector.tensor_tensor(
                                spT[:, 0:PT], spT[:, 0:PT], maskt[:], ALU.add
                            )
                            pts = wk.tile([PT, 4 * PT], BF16, tag="pts", bufs=3)
                            nc.scalar.activation(pts[:, 0:w], spT[:, 0:w], AF.Exp)
                            for idx, r in enumerate(range(d, 4)):
                                nc.tensor.matmul(
                                    yps[r][:], pts[:, idx * PT:(idx + 1) * PT],
                                    v_s[:, j, :],
                                    start=(j == 0), stop=(d == r),
                                )
                        for r in range(4):
                            x0 = (t0r + r) * PT
                            rinv = wk.tile([PT, 1], F32, tag="rinv", bufs=2)
                            nc.vector.reciprocal(rinv[:], yps[r][:, HS:HS + 1])
                            y_sb = wk.tile([PT, HS], BF16, tag="y_sb", bufs=2)
                            nc.vector.tensor_scalar_mul(y_sb[:], yps[r][:, 0:HS], rinv[:])
                            ty = psum.tile([PT, PT], BF16, tag="a", bufs=2)
                            nc.tensor.transpose(ty[:], y_sb[:], ident[:])
                            nc.vector.tensor_copy(yT_s[:, h, x0:x0 + PT], ty[:])
                    if proj_cb is not None:
                        proj_cb(qi)

            if loop_r is None:
                with tc.tile_pool(name="ytfp", bufs=1) as ytfp:
                    done = []

                    def proj_cb(qi):
                        # after quad qi: AG chunks 2qi,2qi+1; proj chunks from
                        # earlier quads (AllGather latency hides behind compute).
                        for c in (2 * qi, 2 * qi + 1):
                            _ag_issue(c)
                        for c in range(NCH):
                            if c in done:
                                continue
                            if c <= 2 * qi - 1 or qi == NT // 4 - 1:
                                ytf = ytfp.tile([PT, NCT, TC], BF16, tag="ytf", bufs=2)
                                nc.sync.dma_start(
                                    ytf[:],
                                    ag_outs[c][:].rearrange("(c2 p) t -> p c2 t", p=PT),
                                )
                                _proj_chunk(c, ytf, local=True)
                                done.append(c)

                    with tc.tile_pool(name="xtp", bufs=1) as xtp:
                        xt_s = xtp.tile([PT, NCT, T], BF16)
                        _load_xt(xt_s)
                        _stage23(xt_s, proj_cb)
            else:
                # timing-only build: loop the whole compute body on-device;
                # proj consumes xt_s (same shape as gathered y^T) - numerics
                # are wrong but per-iteration work matches the real kernel
                # minus the AllGather.
                with tc.tile_pool(name="xtp", bufs=1) as xtp:
                    xt_s = xtp.tile([PT, NCT, T], BF16)

                    def proj_cb(qi):
                        if "P" in ablate:
                            return
                        _proj_chunk(2 * qi, xt_s, local=False)
                        _proj_chunk(2 * qi + 1, xt_s, local=False)

                    with tc.For_i(0, loop_r, 1):
                        _load_xt(xt_s)
                        _stage23(xt_s, proj_cb)

    nc.compile()
    return nc


def _tables(q_scale, k_scale):
    inv_freq = THETA ** (-np.arange(0, HS, 2, dtype=np.float64) / HS)
    ang = np.arange(T, dtype=np.float64)[:, None] * inv_freq[None, :]
    cosw = np.concatenate([np.cos(ang), np.cos(ang)], 1)  # (T, 128)
    sinw = np.concatenate([np.sin(ang), np.sin(ang)], 1)
    qs = np.asarray(q_scale, np.float64)
    ks = np.asarray(k_scale, np.float64)
    qs_rot = np.concatenate([qs[H2:], qs[:H2]])
    ks_rot = np.concatenate([ks[H2:], ks[:H2]])
    s = 1.0 / math.sqrt(HS)
    cosqT = np.ascontiguousarray((cosw * qs[None, :] * s).T).astype(BF)
    sinqT = np.ascontiguousarray((sinw * qs_rot[None, :] * s).T).astype(BF)
    coskT = np.ascontiguousarray((cosw * ks[None, :]).T).astype(BF)
    sinkT = np.ascontiguousarray((sinw * ks_rot[None, :]).T).astype(BF)
    return cosqT, sinqT, coskT, sinkT


def _make_in_maps(x, Wq, Wkv, Wproj, q_scale, k_scale, v1, value_lambda, layer_idx):
    x = np.asarray(x, np.float32)
    Wq = np.asarray(Wq, np.float32)
    Wkv = np.asarray(Wkv, np.float32)
    Wproj = np.asarray(Wproj, np.float32)

    li = int(np.asarray(layer_idx))
    mix = (v1 is not None) and (value_lambda is not None) and li > 0
    lam = float(np.asarray(value_lambda).reshape(())) if mix else 1.0

    cosqT, sinqT, coskT, sinkT = _tables(q_scale, k_scale)
    mneg = (np.tril(np.ones((PT, PT), np.float32), k=-1) * -1e30).astype(np.float32)

    in_maps = []
    for core in range(NCORES):
        b, r = core // TP, core % TP
        kcols = Wkv[:, r * HS:(r + 1) * HS]
        vcols = Wkv[:, NKV * HS + r * HS: NKV * HS + (r + 1) * HS]
        if mix:
            v1s_np = np.ascontiguousarray(
                ((1.0 - lam) * np.asarray(v1, np.float32)[b, :, r, :]).T
            ).astype(np.float32)
        else:
            v1s_np = np.zeros((HS, T), np.float32)
        in_maps.append({
            "xT": np.ascontiguousarray(x[b].T).astype(BF),
            "wq": Wq[:, r * QW:(r + 1) * QW].astype(BF),
            "wkv": np.ascontiguousarray(np.concatenate([kcols, vcols], 1)).astype(BF),
            "wp": np.ascontiguousarray(Wproj[:, r * QW:(r + 1) * QW]).astype(BF),
            "v1sT": v1s_np,
            "cosqT": cosqT, "sinqT": sinqT, "coskT": coskT, "sinkT": sinkT,
            "mneg": mneg,
        })
    return in_maps


def kernel(x, Wq, Wkv, Wproj, q_scale, k_scale, v1, value_lambda, layer_idx):
    in_maps = _make_in_maps(x, Wq, Wkv, Wproj, q_scale, k_scale, v1,
                            value_lambda, layer_idx)
    if "nc" not in _CACHE:
        _CACHE["nc"] = _build()
    nc = _CACHE["nc"]

    trace = bool(int(os.environ.get("BASS_KERNEL_TRACE", "0")))
    res = run_bass_kernel_spmd(nc, in_maps, core_ids=list(range(NCORES)), trace=trace)
    _CACHE["last"] = res

    y = np.empty((B, T, C), np.float32)
    for core in range(NCORES):
        b, r = core // TP, core % TP
        y[b, :, r * QW:(r + 1) * QW] = np.asarray(res.results[core]["out"])
    return y
